# revision 1
# baseline (speedup 1.0000x reference)
"""CIKA conv block on 8 Trainium2 NeuronCores.

Sharding: pure data parallel. 8 shards = (batch n, H half). Each core gets a
zero-padded, W-strip-interleaved bf16 slice of `lower`/`upper` plus
replicated (host-preprocessed) weights, and computes its (32, 128, 256)
slice of both outputs (low, up).

On-chip layout: [128 partitions = 4 W-strips x 32 channels].  Depthwise 5x5
convs run on the TensorEngine as 25 diagonal-matmul taps accumulated in PSUM
(spatial shifts are free AP offsets into the padded SBUF plane).  1x1 convs
are block-diagonal matmuls (kron(I_strips, W^T)).  The dynamic (involution)
conv: a selector matmul replicates each KSA tap plane across the 32 channel
partitions into PSUM; one fused DVE scalar_tensor_tensor forms
m_t = (x_shift * w[c,t]) * ksa_rep in bf16; the tap sum and the following
1x1 are folded into one PSUM accumulation of W_low @ m_t over the 25 taps.
All matmul operands are bf16 (enables PE fast-weight-load); PSUM
accumulation stays fp32.
"""

import os
from contextlib import ExitStack

import numpy as np

import concourse.bacc as bacc
import concourse.bass as bass
import concourse.mybir as mybir
import concourse.tile as tile
from concourse.bass_utils import run_bass_kernel_spmd

F32 = mybir.dt.float32
BF16 = mybir.dt.bfloat16
AF = mybir.ActivationFunctionType
ALU = mybir.AluOpType

KK = 5          # kernel size
CH = 32         # channels
NB, H, W = 4, 256, 256
N_CORES = 8
HSH = H // 2    # rows per core (one batch-half per core)
ROWS_T = 32     # output rows per on-chip tile
NT = HSH // ROWS_T
SW = 64         # strip width (W / 4)
TAPS = [(i, j) for i in range(KK) for j in range(KK)]

LAST_EXEC_NS = None


def _emit(ctx: ExitStack, tc: tile.TileContext, io):
    nc = tc.nc
    (lower_d, upper_d, wdw_d, sel_d, wm1_d, wm2_d, wk1_d, wk2_d, wlow_d,
     wup_d, wdyn_d, bias_d, low_od, up_od) = io

    wpool = ctx.enter_context(tc.tile_pool(name="wts", bufs=1))
    inp = ctx.enter_context(tc.tile_pool(name="inp", bufs=2))
    work = ctx.enter_context(tc.tile_pool(name="work", bufs=2))
    outp = ctx.enter_context(tc.tile_pool(name="outp", bufs=2))
    ps_dw = ctx.enter_context(tc.tile_pool(name="psdw", bufs=2, space="PSUM"))
    ps_pw = ctx.enter_context(tc.tile_pool(name="pspw", bufs=2, space="PSUM"))
    ps_rep = ctx.enter_context(tc.tile_pool(name="psrep", bufs=2,
                                            space="PSUM"))
    ps_low = ctx.enter_context(tc.tile_pool(name="pslow", bufs=2,
                                            space="PSUM"))

    # ---- load weights once (all bf16 except biases) ----
    w_dw = wpool.tile([128, 75, 128], BF16)
    nc.sync.dma_start(w_dw[:], wdw_d[:])
    sel = wpool.tile([128, 25, 128], BF16)
    nc.sync.dma_start(sel[:], sel_d[:])
    w_m1 = wpool.tile([128, 32], BF16)
    nc.sync.dma_start(w_m1[:], wm1_d[:])
    w_m2 = wpool.tile([32, 128], BF16)
    nc.sync.dma_start(w_m2[:], wm2_d[:])
    # w_k1 duplicated on both partition halves so lhsT/rhs slice bases match
    w_k1 = wpool.tile([128, 100], BF16)
    nc.sync.dma_start(w_k1[:], wk1_d[:])
    w_k2 = wpool.tile([100, 64], BF16)
    nc.sync.dma_start(w_k2[:], wk2_d[:])
    w_low = wpool.tile([128, 128], BF16)
    nc.sync.dma_start(w_low[:], wlow_d[:])
    w_up = wpool.tile([128, 128], BF16)
    nc.sync.dma_start(w_up[:], wup_d[:])
    wdyn = wpool.tile([128, 25], F32)
    nc.sync.dma_start(wdyn[:], wdyn_d[:])
    # bias columns [128, 9]: 0 b_kca_dw, 1 b_ksa_dw, 2 b_up_dw, 3 b_m1(32),
    # 4 b_m2, 5 b_k1(100), 6 b_k2(64-slot), 7 b_low, 8 b_up_pw
    bias = wpool.tile([128, 9], F32)
    nc.sync.dma_start(bias[:], bias_d[:])

    def bcol(idx, p=128):
        return bias[0:p, idx:idx + 1]

    # PE can encode only one sync wait per matmul (LDWEIGHTS struct limit).
    # Warm-up matmuls make PE observe every weight-DMA queue once, so real
    # matmuls transitively need no weight waits — just their rhs producer.
    sc = ps_pw.tile([1, 1], F32, tag="pspw")
    for wap in (w_dw[0:1, 0, 0:1], sel[0:1, 0, 0:1], w_m1[0:1, 0:1],
                w_m2[0:1, 0:1], w_k1[0:1, 0:1], w_k2[0:1, 0:1],
                w_low[0:1, 0:1], w_up[0:1, 0:1]):
        nc.tensor.matmul(sc[:], wap, wap, start=True, stop=True)

    for it in range(NT):
        r0 = it * ROWS_T
        low_t = inp.tile([128, ROWS_T + 4, SW + 4], BF16, tag="low_in")
        up_t = inp.tile([128, ROWS_T + 4, SW + 4], BF16, tag="up_in")
        # shards are pre-striped on the host to [128 = strip*32+c, rows, 68]
        nc.sync.dma_start(low_t[:], lower_d[:, r0:r0 + ROWS_T + 4, :])
        nc.sync.dma_start(up_t[:], upper_d[:, r0:r0 + ROWS_T + 4, :])

        # ---- depthwise 5x5 with relu fused in the PSUM evac (ACT) ----
        def dw5(src, cv, out_sb, bias_idx):
            for q in range(4):
                ps = ps_dw.tile([128, 8, SW], F32, tag="psdw")
                for t, (i, j) in enumerate(TAPS):
                    nc.tensor.matmul(
                        ps[:], w_dw[:, cv * 25 + t, :],
                        src[:, q * 8 + i:q * 8 + i + 8, j:j + SW],
                        start=(t == 0), stop=(t == 24))
                nc.scalar.activation(out_sb[:, q * 8:(q + 1) * 8, :], ps[:],
                                     AF.Relu, bias=bcol(bias_idx))

        t_kca = work.tile([128, ROWS_T, SW], BF16, tag="t_kca")
        dw5(low_t, 0, t_kca, 0)

        # ---- KCA chain: 1x1 (32->8) relu, 1x1 (8->32) sigmoid ----
        m1o = work.tile([32, ROWS_T, SW], BF16, tag="m1o")
        for q in range(4):
            ps = ps_pw.tile([32, 8, SW], F32, tag="pspw")
            nc.tensor.matmul(ps[:], w_m1[:], t_kca[:, q * 8:(q + 1) * 8, :],
                             start=True, stop=True)
            nc.scalar.activation(m1o[:, q * 8:(q + 1) * 8, :], ps[:],
                                 AF.Relu, bias=bcol(3, 32))
        kca = work.tile([128, ROWS_T, SW], BF16, tag="kca")
        for q in range(4):
            ps = ps_pw.tile([128, 8, SW], F32, tag="pspw")
            nc.tensor.matmul(ps[:], w_m2[:], m1o[:, q * 8:(q + 1) * 8, :],
                             start=True, stop=True)
            nc.scalar.activation(kca[:, q * 8:(q + 1) * 8, :], ps[:],
                                 AF.Sigmoid, bias=bcol(4))

        # ---- KSA chain (strip pairs: K=64 -> M=100, then K=100 -> M=64) ----
        t_ksa = work.tile([128, ROWS_T, SW], BF16, tag="t_ksa")
        dw5(up_t, 1, t_ksa, 1)
        k1o = work.tile([100, 2, ROWS_T, SW], BF16, tag="k1o")
        for g in range(2):
            for q in range(4):
                ps = ps_pw.tile([100, 8, SW], F32, tag="pspw")
                nc.tensor.matmul(
                    ps[:], w_k1[g * 64:(g + 1) * 64, :],
                    t_ksa[g * 64:(g + 1) * 64, q * 8:(q + 1) * 8, :],
                    start=True, stop=True)
                nc.scalar.activation(k1o[:, g, q * 8:(q + 1) * 8, :], ps[:],
                                     AF.Relu, bias=bcol(5, 100))
        # ksa laid out [128 = strip*32 + tap, rows, cols] (slots 25-31 pad)
        ksa = work.tile([128, ROWS_T, SW], BF16, tag="ksa")
        for g in range(2):
            for q in range(4):
                ps = ps_pw.tile([64, 8, SW], F32, tag="pspw")
                nc.tensor.matmul(ps[:], w_k2[:],
                                 k1o[:, g, q * 8:(q + 1) * 8, :],
                                 start=True, stop=True)
                nc.scalar.activation(
                    ksa[64 * g:64 * (g + 1), q * 8:(q + 1) * 8, :], ps[:],
                    AF.Sigmoid, bias=bcol(6, 64))

        # ---- dynamic conv + low 1x1, fused: low = sum_t W_low @ m_t ----
        # m_t = (lower_shift * w_dyn[c,t]) * ksa_rep[t];  W_low = kron(I4,
        # (low_pw_w * dyn-scale fold is NOT needed — w_dyn applied in STT)
        low_o = outp.tile([128, ROWS_T, SW], F32, tag="low_o")
        for hf in range(2):
            lps0 = ps_low.tile([128, 8, SW], F32, tag="pslow")
            lps1 = ps_low.tile([128, 8, SW], F32, tag="pslow")
            lps = [lps0, lps1]
            for t, (i, j) in enumerate(TAPS):
                for qq in range(2):
                    q = hf * 2 + qq
                    rep = ps_rep.tile([128, 8, SW], F32, tag="rep")
                    nc.tensor.matmul(rep[:], sel[:, t, :],
                                     ksa[:, q * 8:(q + 1) * 8, :],
                                     start=True, stop=True)
                    mt = work.tile([128, 8, SW], BF16, tag=f"mt{qq}")
                    nc.vector.scalar_tensor_tensor(
                        mt[:], low_t[:, q * 8 + i:q * 8 + i + 8, j:j + SW],
                        wdyn[:, t:t + 1], rep[:], ALU.mult, ALU.mult)
                    nc.tensor.matmul(lps[qq][:], w_low[:], mt[:],
                                     start=(t == 0), stop=(t == 24))
            for qq in range(2):
                q = hf * 2 + qq
                nc.vector.tensor_scalar_add(low_o[:, q * 8:(q + 1) * 8, :],
                                            lps[qq][:], bcol(7))

        # ---- up branch: dw5 + bias, gate by kca (fused on DVE), 1x1 ----
        gated = work.tile([128, ROWS_T, SW], BF16, tag="gated")
        for q in range(4):
            ps = ps_dw.tile([128, 8, SW], F32, tag="psdw")
            for t, (i, j) in enumerate(TAPS):
                nc.tensor.matmul(
                    ps[:], w_dw[:, 50 + t, :],
                    up_t[:, q * 8 + i:q * 8 + i + 8, j:j + SW],
                    start=(t == 0), stop=(t == 24))
            nc.vector.scalar_tensor_tensor(
                gated[:, q * 8:(q + 1) * 8, :], ps[:], bcol(2),
                kca[:, q * 8:(q + 1) * 8, :], ALU.add, ALU.mult)
        up_o = outp.tile([128, ROWS_T, SW], F32, tag="up_o")
        for q in range(4):
            ps = ps_pw.tile([128, 8, SW], F32, tag="pspw")
            nc.tensor.matmul(ps[:], w_up[:], gated[:, q * 8:(q + 1) * 8, :],
                             start=True, stop=True)
            nc.vector.tensor_scalar_add(up_o[:, q * 8:(q + 1) * 8, :],
                                        ps[:], bcol(8))

        # ---- store (striped layout; host de-interleaves) ----
        nc.sync.dma_start(low_od[:, r0:r0 + ROWS_T, :], low_o[:])
        nc.sync.dma_start(up_od[:, r0:r0 + ROWS_T, :], up_o[:])


_NC_CACHE = {}


def _build_nc():
    if "nc" in _NC_CACHE:
        return _NC_CACHE["nc"]
    nc = bacc.Bacc("TRN2", target_bir_lowering=False)
    lower_d = nc.dram_tensor("lower_sh", (128, HSH + 4, SW + 4), BF16,
                             kind="ExternalInput")
    upper_d = nc.dram_tensor("upper_sh", (128, HSH + 4, SW + 4), BF16,
                             kind="ExternalInput")
    wdw_d = nc.dram_tensor("w_dw", (128, 75, 128), BF16, kind="ExternalInput")
    sel_d = nc.dram_tensor("sel", (128, 25, 128), BF16, kind="ExternalInput")
    wm1_d = nc.dram_tensor("w_m1", (128, 32), BF16, kind="ExternalInput")
    wm2_d = nc.dram_tensor("w_m2", (32, 128), BF16, kind="ExternalInput")
    wk1_d = nc.dram_tensor("w_k1", (128, 100), BF16, kind="ExternalInput")
    wk2_d = nc.dram_tensor("w_k2", (100, 64), BF16, kind="ExternalInput")
    wlow_d = nc.dram_tensor("w_low", (128, 128), BF16, kind="ExternalInput")
    wup_d = nc.dram_tensor("w_up", (128, 128), BF16, kind="ExternalInput")
    wdyn_d = nc.dram_tensor("w_dyn", (128, 25), F32, kind="ExternalInput")
    bias_d = nc.dram_tensor("biases", (128, 9), F32, kind="ExternalInput")
    low_od = nc.dram_tensor("low_out", (128, HSH, SW), F32,
                            kind="ExternalOutput")
    up_od = nc.dram_tensor("up_out", (128, HSH, SW), F32,
                           kind="ExternalOutput")
    io = (lower_d, upper_d, wdw_d, sel_d, wm1_d, wm2_d, wk1_d, wk2_d,
          wlow_d, wup_d, wdyn_d, bias_d, low_od, up_od)
    with tile.TileContext(nc) as tc:
        with ExitStack() as ctx:
            _emit(ctx, tc, io)
    nc.compile()
    _NC_CACHE["nc"] = nc
    return nc


def _prep_weights(kca_dw_w, kca_dw_b, kca_m1_w, kca_m1_b, kca_m2_w, kca_m2_b,
                  ksa_dw_w, ksa_dw_b, ksa_m1_w, ksa_m1_b, ksa_m2_w, ksa_m2_b,
                  low_dyn_w, low_dyn_b, low_pw_w, low_pw_b,
                  up_dw_w, up_dw_b, up_pw_w, up_pw_b):
    f = np.float32
    import ml_dtypes
    bf = ml_dtypes.bfloat16
    w_dw = np.zeros((128, 75, 128), f)
    ar = np.arange(128)
    for cv, wt in enumerate([kca_dw_w, ksa_dw_w, up_dw_w]):
        w2 = np.asarray(wt, f).reshape(CH, 25)
        for t in range(25):
            w_dw[ar, cv * 25 + t, ar] = np.tile(w2[:, t], 4)
    sel = np.zeros((128, 25, 128), f)
    for s in range(4):
        for t in range(25):
            sel[s * 32 + t, t, s * 32:(s + 1) * 32] = 1.0
    i4, i2 = np.eye(4, dtype=f), np.eye(2, dtype=f)
    w_m1 = np.kron(i4, np.asarray(kca_m1_w, f).T)        # (128, 32)
    w_m2 = np.kron(i4, np.asarray(kca_m2_w, f).T)        # (32, 128)
    w_k1 = np.kron(i2, np.asarray(ksa_m1_w, f).T)        # (64, 100)
    w_k1 = np.vstack([w_k1, w_k1])                       # (128, 100) dup
    w_k2 = np.zeros((100, 64), f)                        # padded to 32-slots
    w2t = np.asarray(ksa_m2_w, f).T                      # (50, 25)
    for sl in range(2):
        w_k2[sl * 50:(sl + 1) * 50, sl * 32:sl * 32 + 25] = w2t
    w_low = np.kron(i4, np.asarray(low_pw_w, f).T)       # (128, 128)
    w_up = np.kron(i4, np.asarray(up_pw_w, f).T)         # (128, 128)
    w_dyn = np.tile(np.asarray(low_dyn_w, f).reshape(CH, 25), (4, 1))
    bias = np.zeros((128, 9), f)
    bias[:, 0] = np.tile(np.asarray(kca_dw_b, f), 4)
    bias[:, 1] = np.tile(np.asarray(ksa_dw_b, f), 4)
    bias[:, 2] = np.tile(np.asarray(up_dw_b, f), 4)
    bias[:32, 3] = np.tile(np.asarray(kca_m1_b, f), 4)
    bias[:, 4] = np.tile(np.asarray(kca_m2_b, f), 4)
    bias[:100, 5] = np.tile(np.asarray(ksa_m1_b, f), 2)
    for sl in range(2):
        bias[sl * 32:sl * 32 + 25, 6] = np.asarray(ksa_m2_b, f)
    b_low = np.asarray(low_pw_w, f) @ np.asarray(low_dyn_b, f).reshape(CH) \
        + np.asarray(low_pw_b, f)
    bias[:, 7] = np.tile(b_low, 4)
    bias[:, 8] = np.tile(np.asarray(up_pw_b, f), 4)
    return dict(w_dw=w_dw.astype(bf), sel=sel.astype(bf),
                w_m1=w_m1.astype(bf), w_m2=w_m2.astype(bf),
                w_k1=w_k1.astype(bf), w_k2=w_k2.astype(bf),
                w_low=w_low.astype(bf), w_up=w_up.astype(bf),
                w_dyn=w_dyn, biases=bias)


def kernel(lower, upper, **wts):
    global LAST_EXEC_NS
    import ml_dtypes
    bf = ml_dtypes.bfloat16
    nc = _build_nc()
    wmap = _prep_weights(**wts)
    lp = np.pad(np.ascontiguousarray(np.asarray(lower, np.float32)),
                ((0, 0), (0, 0), (2, 2), (2, 2))).astype(bf)
    up = np.pad(np.ascontiguousarray(np.asarray(upper, np.float32)),
                ((0, 0), (0, 0), (2, 2), (2, 2))).astype(bf)

    def stripe(x):
        # (32, 132, 260) -> (128 = strip*32+c, 132, 68), strips overlap by 4
        out = np.empty((128, HSH + 4, SW + 4), bf)
        for s in range(4):
            out[s * 32:(s + 1) * 32] = x[:, :, s * SW:s * SW + SW + 4]
        return out

    in_maps = []
    for k in range(N_CORES):
        n, half = k // 2, k % 2
        m = dict(wmap)
        m["lower_sh"] = stripe(lp[n, :, half * HSH:half * HSH + HSH + 4, :])
        m["upper_sh"] = stripe(up[n, :, half * HSH:half * HSH + HSH + 4, :])
        in_maps.append(m)
    trace = os.environ.get("BASS_KERNEL_TRACE", "0") == "1"
    res = run_bass_kernel_spmd(nc, in_maps, core_ids=list(range(N_CORES)),
                               trace=trace)
    LAST_EXEC_NS = res.exec_time_ns
    low = np.empty((NB, CH, H, W), np.float32)
    upo = np.empty((NB, CH, H, W), np.float32)
    for k in range(N_CORES):
        n, half = k // 2, k % 2
        for s in range(4):
            low[n, :, half * HSH:(half + 1) * HSH, s * SW:(s + 1) * SW] = \
                res.results[k]["low_out"][s * 32:(s + 1) * 32]
            upo[n, :, half * HSH:(half + 1) * HSH, s * SW:(s + 1) * SW] = \
                res.results[k]["up_out"][s * 32:(s + 1) * 32]
    return low, upo



# revision 15
# speedup vs baseline: 1.0049x; 1.0049x over previous
"""CIKA conv block on 8 Trainium2 NeuronCores.

Sharding: pure data parallel. 8 shards = (batch n, H half). Each core gets a
zero-padded, W-strip-interleaved bf16 slice of `lower`/`upper` plus
replicated (host-preprocessed) weights, and computes its (32, 128, 256)
slice of both outputs (low, up).

On-chip layout: [128 partitions = 4 W-strips x 32 channels].  Depthwise 5x5
convs run on the TensorEngine as 25 diagonal-matmul taps accumulated in PSUM
(spatial shifts are free AP offsets into the padded SBUF plane).  1x1 convs
are block-diagonal matmuls (kron(I_strips, W^T)).  The dynamic (involution)
conv: a selector matmul replicates each KSA tap plane across the 32 channel
partitions into PSUM; one fused DVE scalar_tensor_tensor forms
m_t = (x_shift * w[c,t]) * ksa_rep in bf16; the tap sum and the following
1x1 are folded into one PSUM accumulation of W_low @ m_t over the 25 taps.
All matmul operands are bf16 (enables PE fast-weight-load); PSUM
accumulation stays fp32.
"""

import os
from contextlib import ExitStack

import numpy as np

import concourse.bacc as bacc
import concourse.bass as bass
import concourse.mybir as mybir
import concourse.tile as tile
from concourse.bass_utils import run_bass_kernel_spmd

F32 = mybir.dt.float32
BF16 = mybir.dt.bfloat16
AF = mybir.ActivationFunctionType
ALU = mybir.AluOpType

KK = 5          # kernel size
CH = 32         # channels
NB, H, W = 4, 256, 256
N_CORES = 8
HSH = H // 2    # rows per core (one batch-half per core)
ROWS_T = 32     # output rows per on-chip tile
NT = HSH // ROWS_T
SW = 64         # strip width (W / 4)
TAPS = [(i, j) for i in range(KK) for j in range(KK)]

LAST_EXEC_NS = None


def _emit(ctx: ExitStack, tc: tile.TileContext, io):
    nc = tc.nc
    (lower_d, upper_d, wdw_d, sel_d, wm1_d, wm2_d, wk1_d, wk2_d, wlow_d,
     wup_d, wdyn_d, bias_d, low_od, up_od) = io

    wpool = ctx.enter_context(tc.tile_pool(name="wts", bufs=1))
    inp = ctx.enter_context(tc.tile_pool(name="inp", bufs=2))
    work = ctx.enter_context(tc.tile_pool(name="work", bufs=2))
    outp = ctx.enter_context(tc.tile_pool(name="outp", bufs=2))
    ps_dw = ctx.enter_context(tc.tile_pool(name="psdw", bufs=2, space="PSUM"))
    ps_pw = ctx.enter_context(tc.tile_pool(name="pspw", bufs=2, space="PSUM"))
    ps_rep = ctx.enter_context(tc.tile_pool(name="psrep", bufs=2,
                                            space="PSUM"))
    ps_low = ctx.enter_context(tc.tile_pool(name="pslow", bufs=2,
                                            space="PSUM"))

    # ---- load weights once (all bf16 except biases) ----
    w_dw = wpool.tile([128, 75, 128], BF16)
    nc.sync.dma_start(w_dw[:], wdw_d[:])
    sel = wpool.tile([128, 25, 128], BF16)
    nc.sync.dma_start(sel[:], sel_d[:])
    w_m1 = wpool.tile([128, 32], BF16)
    nc.sync.dma_start(w_m1[:], wm1_d[:])
    w_m2 = wpool.tile([32, 128], BF16)
    nc.sync.dma_start(w_m2[:], wm2_d[:])
    # w_k1 duplicated on both partition halves so lhsT/rhs slice bases match
    w_k1 = wpool.tile([128, 100], BF16)
    nc.sync.dma_start(w_k1[:], wk1_d[:])
    w_k2 = wpool.tile([100, 64], BF16)
    nc.sync.dma_start(w_k2[:], wk2_d[:])
    w_low = wpool.tile([128, 128], BF16)
    nc.sync.dma_start(w_low[:], wlow_d[:])
    w_up = wpool.tile([128, 128], BF16)
    nc.sync.dma_start(w_up[:], wup_d[:])
    wdyn = wpool.tile([128, 25], F32)
    nc.sync.dma_start(wdyn[:], wdyn_d[:])
    # bias columns [128, 9]: 0 b_kca_dw, 1 b_ksa_dw, 2 b_up_dw, 3 b_m1(32),
    # 4 b_m2, 5 b_k1(100), 6 b_k2(64-slot), 7 b_low, 8 b_up_pw
    bias = wpool.tile([128, 9], F32)
    nc.sync.dma_start(bias[:], bias_d[:])

    def bcol(idx, p=128):
        return bias[0:p, idx:idx + 1]

    # PE can encode only one sync wait per matmul (LDWEIGHTS struct limit).
    # Warm-up matmuls make PE observe every weight-DMA queue once, so real
    # matmuls transitively need no weight waits — just their rhs producer.
    sc = ps_pw.tile([1, 1], F32, tag="pspw")
    for wap in (w_dw[0:1, 0, 0:1], sel[0:1, 0, 0:1], w_m1[0:1, 0:1],
                w_m2[0:1, 0:1], w_k1[0:1, 0:1], w_k2[0:1, 0:1],
                w_low[0:1, 0:1], w_up[0:1, 0:1]):
        nc.tensor.matmul(sc[:], wap, wap, start=True, stop=True)

    for it in range(NT):
        r0 = it * ROWS_T
        low_t = inp.tile([128, ROWS_T + 4, SW + 4], BF16, tag="low_in")
        up_t = inp.tile([128, ROWS_T + 4, SW + 4], BF16, tag="up_in")
        # shards are pre-striped on the host to [128 = strip*32+c, rows, 68]
        nc.sync.dma_start(low_t[:], lower_d[:, r0:r0 + ROWS_T + 4, :])
        nc.sync.dma_start(up_t[:], upper_d[:, r0:r0 + ROWS_T + 4, :])

        # ---- depthwise 5x5 with relu fused in the PSUM evac (ACT) ----
        def dw5(src, cv, out_sb, bias_idx):
            for q in range(4):
                ps = ps_dw.tile([128, 8, SW], F32, tag="psdw")
                for t, (i, j) in enumerate(TAPS):
                    nc.tensor.matmul(
                        ps[:], w_dw[:, cv * 25 + t, :],
                        src[:, q * 8 + i:q * 8 + i + 8, j:j + SW],
                        start=(t == 0), stop=(t == 24))
                nc.scalar.activation(out_sb[:, q * 8:(q + 1) * 8, :], ps[:],
                                     AF.Relu, bias=bcol(bias_idx))

        t_kca = work.tile([128, ROWS_T, SW], BF16, tag="t_kca")
        dw5(low_t, 0, t_kca, 0)

        # ---- KCA chain: 1x1 (32->8) relu, 1x1 (8->32) sigmoid ----
        m1o = work.tile([32, ROWS_T, SW], BF16, tag="m1o")
        for q in range(4):
            ps = ps_pw.tile([32, 8, SW], F32, tag="pspw")
            nc.tensor.matmul(ps[:], w_m1[:], t_kca[:, q * 8:(q + 1) * 8, :],
                             start=True, stop=True)
            nc.scalar.activation(m1o[:, q * 8:(q + 1) * 8, :], ps[:],
                                 AF.Relu, bias=bcol(3, 32))
        kca = work.tile([128, ROWS_T, SW], BF16, tag="kca")
        for q in range(4):
            ps = ps_pw.tile([128, 8, SW], F32, tag="pspw")
            nc.tensor.matmul(ps[:], w_m2[:], m1o[:, q * 8:(q + 1) * 8, :],
                             start=True, stop=True)
            nc.scalar.activation(kca[:, q * 8:(q + 1) * 8, :], ps[:],
                                 AF.Sigmoid, bias=bcol(4))

        # ---- KSA chain (strip pairs: K=64 -> M=100, then K=100 -> M=64) ----
        t_ksa = work.tile([128, ROWS_T, SW], BF16, tag="t_ksa")
        dw5(up_t, 1, t_ksa, 1)
        k1o = work.tile([100, 2, ROWS_T, SW], BF16, tag="k1o")
        for g in range(2):
            for q in range(4):
                ps = ps_pw.tile([100, 8, SW], F32, tag="pspw")
                nc.tensor.matmul(
                    ps[:], w_k1[g * 64:(g + 1) * 64, :],
                    t_ksa[g * 64:(g + 1) * 64, q * 8:(q + 1) * 8, :],
                    start=True, stop=True)
                nc.scalar.activation(k1o[:, g, q * 8:(q + 1) * 8, :], ps[:],
                                     AF.Relu, bias=bcol(5, 100))
        # ksa laid out [128 = strip*32 + tap, rows, cols] (slots 25-31 pad)
        ksa = work.tile([128, ROWS_T, SW], BF16, tag="ksa")
        for g in range(2):
            for q in range(4):
                ps = ps_pw.tile([64, 8, SW], F32, tag="pspw")
                nc.tensor.matmul(ps[:], w_k2[:],
                                 k1o[:, g, q * 8:(q + 1) * 8, :],
                                 start=True, stop=True)
                nc.scalar.activation(
                    ksa[64 * g:64 * (g + 1), q * 8:(q + 1) * 8, :], ps[:],
                    AF.Sigmoid, bias=bcol(6, 64))

        # ---- dynamic conv + low 1x1, fused: low = sum_t W_low @ m_t ----
        # m_t = (lower_shift * w_dyn[c,t]) * ksa_rep[t];  W_low = kron(I4,
        # (low_pw_w * dyn-scale fold is NOT needed — w_dyn applied in STT)
        low_o = outp.tile([128, ROWS_T, SW], F32, tag="low_o")
        for hf in range(2):
            lps0 = ps_low.tile([128, 8, SW], F32, tag="pslow")
            lps1 = ps_low.tile([128, 8, SW], F32, tag="pslow")
            lps = [lps0, lps1]
            for t, (i, j) in enumerate(TAPS):
                for qq in range(2):
                    q = hf * 2 + qq
                    rep = ps_rep.tile([128, 8, SW], F32, tag="rep")
                    nc.tensor.matmul(rep[:], sel[:, t, :],
                                     ksa[:, q * 8:(q + 1) * 8, :],
                                     start=True, stop=True)
                    mt = work.tile([128, 8, SW], BF16, tag=f"mt{qq}")
                    nc.vector.scalar_tensor_tensor(
                        mt[:], low_t[:, q * 8 + i:q * 8 + i + 8, j:j + SW],
                        wdyn[:, t:t + 1], rep[:], ALU.mult, ALU.mult)
                    nc.tensor.matmul(lps[qq][:], w_low[:], mt[:],
                                     start=(t == 0), stop=(t == 24))
            for qq in range(2):
                q = hf * 2 + qq
                nc.vector.tensor_scalar_add(low_o[:, q * 8:(q + 1) * 8, :],
                                            lps[qq][:], bcol(7))

        # ---- up branch: dw5 + bias, gate by kca (fused on DVE), 1x1 ----
        gated = work.tile([128, ROWS_T, SW], BF16, tag="gated")
        for q in range(4):
            ps = ps_dw.tile([128, 8, SW], F32, tag="psdw")
            for t, (i, j) in enumerate(TAPS):
                nc.tensor.matmul(
                    ps[:], w_dw[:, 50 + t, :],
                    up_t[:, q * 8 + i:q * 8 + i + 8, j:j + SW],
                    start=(t == 0), stop=(t == 24))
            nc.vector.scalar_tensor_tensor(
                gated[:, q * 8:(q + 1) * 8, :], ps[:], bcol(2),
                kca[:, q * 8:(q + 1) * 8, :], ALU.add, ALU.mult)
        up_o = outp.tile([128, ROWS_T, SW], F32, tag="up_o")
        for q in range(4):
            ps = ps_pw.tile([128, 8, SW], F32, tag="pspw")
            nc.tensor.matmul(ps[:], w_up[:], gated[:, q * 8:(q + 1) * 8, :],
                             start=True, stop=True)
            nc.vector.tensor_scalar_add(up_o[:, q * 8:(q + 1) * 8, :],
                                        ps[:], bcol(8))

        # ---- store (striped layout; host de-interleaves) ----
        nc.sync.dma_start(low_od[:, r0:r0 + ROWS_T, :], low_o[:])
        nc.sync.dma_start(up_od[:, r0:r0 + ROWS_T, :], up_o[:])


_NC_CACHE = {}


def _build_nc():
    if "nc" in _NC_CACHE:
        return _NC_CACHE["nc"]
    nc = bacc.Bacc("TRN2", target_bir_lowering=False)
    lower_d = nc.dram_tensor("lower_sh", (128, HSH + 4, SW + 4), BF16,
                             kind="ExternalInput")
    upper_d = nc.dram_tensor("upper_sh", (128, HSH + 4, SW + 4), BF16,
                             kind="ExternalInput")
    wdw_d = nc.dram_tensor("w_dw", (128, 75, 128), BF16, kind="ExternalInput")
    sel_d = nc.dram_tensor("sel", (128, 25, 128), BF16, kind="ExternalInput")
    wm1_d = nc.dram_tensor("w_m1", (128, 32), BF16, kind="ExternalInput")
    wm2_d = nc.dram_tensor("w_m2", (32, 128), BF16, kind="ExternalInput")
    wk1_d = nc.dram_tensor("w_k1", (128, 100), BF16, kind="ExternalInput")
    wk2_d = nc.dram_tensor("w_k2", (100, 64), BF16, kind="ExternalInput")
    wlow_d = nc.dram_tensor("w_low", (128, 128), BF16, kind="ExternalInput")
    wup_d = nc.dram_tensor("w_up", (128, 128), BF16, kind="ExternalInput")
    wdyn_d = nc.dram_tensor("w_dyn", (128, 25), F32, kind="ExternalInput")
    bias_d = nc.dram_tensor("biases", (128, 9), F32, kind="ExternalInput")
    low_od = nc.dram_tensor("low_out", (128, HSH, SW), F32,
                            kind="ExternalOutput")
    up_od = nc.dram_tensor("up_out", (128, HSH, SW), F32,
                           kind="ExternalOutput")
    io = (lower_d, upper_d, wdw_d, sel_d, wm1_d, wm2_d, wk1_d, wk2_d,
          wlow_d, wup_d, wdyn_d, bias_d, low_od, up_od)
    with tile.TileContext(nc) as tc:
        with ExitStack() as ctx:
            _emit(ctx, tc, io)
    nc.compile()
    _NC_CACHE["nc"] = nc
    return nc


def _prep_weights(kca_dw_w, kca_dw_b, kca_m1_w, kca_m1_b, kca_m2_w, kca_m2_b,
                  ksa_dw_w, ksa_dw_b, ksa_m1_w, ksa_m1_b, ksa_m2_w, ksa_m2_b,
                  low_dyn_w, low_dyn_b, low_pw_w, low_pw_b,
                  up_dw_w, up_dw_b, up_pw_w, up_pw_b):
    f = np.float32
    import ml_dtypes
    bf = ml_dtypes.bfloat16
    w_dw = np.zeros((128, 75, 128), f)
    ar = np.arange(128)
    for cv, wt in enumerate([kca_dw_w, ksa_dw_w, up_dw_w]):
        w2 = np.asarray(wt, f).reshape(CH, 25)
        for t in range(25):
            w_dw[ar, cv * 25 + t, ar] = np.tile(w2[:, t], 4)
    sel = np.zeros((128, 25, 128), f)
    for s in range(4):
        for t in range(25):
            sel[s * 32 + t, t, s * 32:(s + 1) * 32] = 1.0
    i4, i2 = np.eye(4, dtype=f), np.eye(2, dtype=f)
    w_m1 = np.kron(i4, np.asarray(kca_m1_w, f).T)        # (128, 32)
    w_m2 = np.kron(i4, np.asarray(kca_m2_w, f).T)        # (32, 128)
    w_k1 = np.kron(i2, np.asarray(ksa_m1_w, f).T)        # (64, 100)
    w_k1 = np.vstack([w_k1, w_k1])                       # (128, 100) dup
    w_k2 = np.zeros((100, 64), f)                        # padded to 32-slots
    w2t = np.asarray(ksa_m2_w, f).T                      # (50, 25)
    for sl in range(2):
        w_k2[sl * 50:(sl + 1) * 50, sl * 32:sl * 32 + 25] = w2t
    w_low = np.kron(i4, np.asarray(low_pw_w, f).T)       # (128, 128)
    w_up = np.kron(i4, np.asarray(up_pw_w, f).T)         # (128, 128)
    w_dyn = np.tile(np.asarray(low_dyn_w, f).reshape(CH, 25), (4, 1))
    bias = np.zeros((128, 9), f)
    bias[:, 0] = np.tile(np.asarray(kca_dw_b, f), 4)
    bias[:, 1] = np.tile(np.asarray(ksa_dw_b, f), 4)
    bias[:, 2] = np.tile(np.asarray(up_dw_b, f), 4)
    bias[:32, 3] = np.tile(np.asarray(kca_m1_b, f), 4)
    bias[:, 4] = np.tile(np.asarray(kca_m2_b, f), 4)
    bias[:100, 5] = np.tile(np.asarray(ksa_m1_b, f), 2)
    for sl in range(2):
        bias[sl * 32:sl * 32 + 25, 6] = np.asarray(ksa_m2_b, f)
    b_low = np.asarray(low_pw_w, f) @ np.asarray(low_dyn_b, f).reshape(CH) \
        + np.asarray(low_pw_b, f)
    bias[:, 7] = np.tile(b_low, 4)
    bias[:, 8] = np.tile(np.asarray(up_pw_b, f), 4)
    return dict(w_dw=w_dw.astype(bf), sel=sel.astype(bf),
                w_m1=w_m1.astype(bf), w_m2=w_m2.astype(bf),
                w_k1=w_k1.astype(bf), w_k2=w_k2.astype(bf),
                w_low=w_low.astype(bf), w_up=w_up.astype(bf),
                w_dyn=w_dyn, biases=bias)


def kernel(lower, upper, **wts):
    global LAST_EXEC_NS
    import ml_dtypes
    bf = ml_dtypes.bfloat16
    nc = _build_nc()
    wmap = _prep_weights(**wts)
    lp = np.pad(np.ascontiguousarray(np.asarray(lower, np.float32)),
                ((0, 0), (0, 0), (2, 2), (2, 2))).astype(bf)
    up = np.pad(np.ascontiguousarray(np.asarray(upper, np.float32)),
                ((0, 0), (0, 0), (2, 2), (2, 2))).astype(bf)

    def stripe(x):
        # (32, 132, 260) -> (128 = strip*32+c, 132, 68), strips overlap by 4
        out = np.empty((128, HSH + 4, SW + 4), bf)
        for s in range(4):
            out[s * 32:(s + 1) * 32] = x[:, :, s * SW:s * SW + SW + 4]
        return out

    in_maps = []
    for k in range(N_CORES):
        n, half = k // 2, k % 2
        m = dict(wmap)
        m["lower_sh"] = stripe(lp[n, :, half * HSH:half * HSH + HSH + 4, :])
        m["upper_sh"] = stripe(up[n, :, half * HSH:half * HSH + HSH + 4, :])
        in_maps.append(m)
    trace = os.environ.get("BASS_KERNEL_TRACE", "0") == "1"
    res = run_bass_kernel_spmd(nc, in_maps, core_ids=list(range(N_CORES)),
                               trace=trace)
    LAST_EXEC_NS = res.exec_time_ns
    low = np.empty((NB, CH, H, W), np.float32)
    upo = np.empty((NB, CH, H, W), np.float32)
    for k in range(N_CORES):
        n, half = k // 2, k % 2
        for s in range(4):
            low[n, :, half * HSH:(half + 1) * HSH, s * SW:(s + 1) * SW] = \
                res.results[k]["low_out"][s * 32:(s + 1) * 32]
            upo[n, :, half * HSH:(half + 1) * HSH, s * SW:(s + 1) * SW] = \
                res.results[k]["up_out"][s * 32:(s + 1) * 32]
    return low, upo



# revision 24
# speedup vs baseline: 1.1445x; 1.1390x over previous
"""CIKA conv block on 8 Trainium2 NeuronCores.

Sharding: pure data parallel. 8 shards = (batch n, H half). Each core gets a
zero-padded, W-strip-interleaved bf16 slice of `lower`/`upper` plus
replicated (host-preprocessed) weights, and computes its (32, 128, 256)
slice of both outputs (low, up).

On-chip layout: [128 partitions = 4 W-strips x 32 channels].  Depthwise 5x5
convs run on the TensorEngine as 25 diagonal-matmul taps accumulated in PSUM
(spatial shifts are free AP offsets into the padded SBUF plane).  1x1 convs
are block-diagonal matmuls (kron(I_strips, W^T)).  The dynamic (involution)
conv: a selector matmul replicates each KSA tap plane across the 32 channel
partitions into PSUM; one fused DVE scalar_tensor_tensor forms
m_t = (x_shift * w[c,t]) * ksa_rep in bf16; the tap sum and the following
1x1 are folded into one PSUM accumulation of W_low @ m_t over the 25 taps.
All matmul operands are bf16 (enables PE fast-weight-load); PSUM
accumulation stays fp32.
"""

import os
from contextlib import ExitStack

import numpy as np

import bass_rust
import concourse.bacc as bacc
import concourse.bass as bass
import concourse.mybir as mybir
import concourse.tile as tile
from concourse.bass_utils import run_bass_kernel_spmd

F32 = mybir.dt.float32
BF16 = mybir.dt.bfloat16
F8 = mybir.dt.float8e4
DR = mybir.MatmulPerfMode.DoubleRow
AF = mybir.ActivationFunctionType
ALU = mybir.AluOpType

KK = 5          # kernel size
CH = 32         # channels
NB, H, W = 4, 256, 256
N_CORES = 8
HSH = H // 2    # rows per core (one batch-half per core)
ROWS_T = 32     # output rows per on-chip tile
NT = HSH // ROWS_T
SW = 64         # strip width (W / 4)
TAPS = [(i, j) for i in range(KK) for j in range(KK)]

LAST_EXEC_NS = None


def _emit(ctx: ExitStack, tc: tile.TileContext, io):
    nc = tc.nc
    (lower_d, upper_d, lower8_d, upper8_d, wdw_d, wdw8_d, sel_d, wm1_d,
     wm2_d, wk1_d, wk2_d, wlow_d, wup_d, wdyn_d, bias_d, low_od, up_od) = io

    wpool = ctx.enter_context(tc.tile_pool(name="wts", bufs=1))
    inp = ctx.enter_context(tc.tile_pool(name="inp", bufs=2))
    work = ctx.enter_context(tc.tile_pool(name="work", bufs=2))
    outp = ctx.enter_context(tc.tile_pool(name="outp", bufs=2))
    ps_dw = ctx.enter_context(tc.tile_pool(name="psdw", bufs=2, space="PSUM"))
    ps_pw = ctx.enter_context(tc.tile_pool(name="pspw", bufs=2, space="PSUM"))
    ps_rep = ctx.enter_context(tc.tile_pool(name="psrep", bufs=2,
                                            space="PSUM"))
    ps_low = ctx.enter_context(tc.tile_pool(name="pslow", bufs=2,
                                            space="PSUM"))

    # ---- load weights once (all bf16 except biases) ----
    w_dw = wpool.tile([128, 75, 128], BF16)
    nc.sync.dma_start(w_dw[:], wdw_d[:])
    # fp8 DoubleRow tap-pair weights for the two gate-path dw5 convs.
    # Vertical pairs (k-tile delta = row stride): per col j, row pairs
    # (0,1), (2,3), (4,zero) -> 15 pairs. [128, cv*30 + pair*2 + kt, 128]
    w_dw8 = wpool.tile([128, 60, 128], F8)
    nc.sync.dma_start(w_dw8[:], wdw8_d[:])
    sel = wpool.tile([128, 25, 128], BF16)
    nc.sync.dma_start(sel[:], sel_d[:])
    w_m1 = wpool.tile([128, 32], BF16)
    nc.sync.dma_start(w_m1[:], wm1_d[:])
    w_m2 = wpool.tile([32, 128], BF16)
    nc.sync.dma_start(w_m2[:], wm2_d[:])
    # w_k1 duplicated on both partition halves so lhsT/rhs slice bases match
    w_k1 = wpool.tile([128, 100], BF16)
    nc.sync.dma_start(w_k1[:], wk1_d[:])
    w_k2 = wpool.tile([100, 64], BF16)
    nc.sync.dma_start(w_k2[:], wk2_d[:])
    w_low = wpool.tile([128, 128], BF16)
    nc.sync.dma_start(w_low[:], wlow_d[:])
    w_up = wpool.tile([128, 128], BF16)
    nc.sync.dma_start(w_up[:], wup_d[:])
    wdyn = wpool.tile([128, 25], F32)
    nc.sync.dma_start(wdyn[:], wdyn_d[:])
    # bias columns [128, 9]: 0 b_kca_dw, 1 b_ksa_dw, 2 b_up_dw, 3 b_m1(32),
    # 4 b_m2, 5 b_k1(100), 6 b_k2(64-slot), 7 b_low, 8 b_up_pw
    bias = wpool.tile([128, 9], F32)
    nc.sync.dma_start(bias[:], bias_d[:])

    def bcol(idx, p=128):
        return bias[0:p, idx:idx + 1]

    # PE can encode only one sync wait per matmul (LDWEIGHTS struct limit).
    # Warm-up matmuls make PE observe every weight-DMA queue once, so real
    # matmuls transitively need no weight waits — just their rhs producer.
    sc = ps_pw.tile([1, 1], F32, tag="pspw")
    for wap in (w_dw[0:1, 0, 0:1], w_dw8[0:1, 0, 0:1], sel[0:1, 0, 0:1],
                w_m1[0:1, 0:1], w_m2[0:1, 0:1], w_k1[0:1, 0:1],
                w_k2[0:1, 0:1], w_low[0:1, 0:1], w_up[0:1, 0:1]):
        nc.tensor.matmul(sc[:], wap, wap, start=True, stop=True)

    for it in range(NT):
        r0 = it * ROWS_T
        low_t = inp.tile([128, ROWS_T + 4, SW + 4], BF16, tag="low_in")
        up_t = inp.tile([128, ROWS_T + 4, SW + 4], BF16, tag="up_in")
        low8_t = inp.tile([128, ROWS_T + 5, SW + 4], F8, tag="low8_in")
        up8_t = inp.tile([128, ROWS_T + 5, SW + 4], F8, tag="up8_in")
        # shards are pre-striped on the host to [128 = strip*32+c, rows, 68]
        nc.sync.dma_start(low_t[:], lower_d[:, r0:r0 + ROWS_T + 4, :])
        nc.sync.dma_start(up_t[:], upper_d[:, r0:r0 + ROWS_T + 4, :])
        nc.sync.dma_start(low8_t[:], lower8_d[:, r0:r0 + ROWS_T + 5, :])
        nc.sync.dma_start(up8_t[:], upper8_d[:, r0:r0 + ROWS_T + 5, :])

        # ---- depthwise 5x5 with relu fused in the PSUM evac (ACT) ----
        def dw5(src, cv, out_sb, bias_idx):
            for q in range(4):
                ps = ps_dw.tile([128, 8, SW], F32, tag="psdw")
                for t, (i, j) in enumerate(TAPS):
                    nc.tensor.matmul(
                        ps[:], w_dw[:, cv * 25 + t, :],
                        src[:, q * 8 + i:q * 8 + i + 8, j:j + SW],
                        start=(t == 0), stop=(t == 24))
                nc.scalar.activation(out_sb[:, q * 8:(q + 1) * 8, :], ps[:],
                                     AF.Relu, bias=bcol(bias_idx))

        def dr_rhs(src8, q, i0, j):
            # [128, 2 (vertical tap-pair k-tiles, delta = row stride), 8, 64]
            base = src8[:, q * 8 + i0:q * 8 + i0 + 8, j:j + SW]
            raw = [list(d) for d in base.ap]
            return bass_rust.AP(
                base.tensor, base.offset,
                [raw[0], [SW + 4, 2], raw[1], raw[2]])

        # gate-path dw5: 15 fp8 DoubleRow vertical tap-pairs per q-chunk
        # (per col j: row pairs (0,1), (2,3), (4,zero))
        def dw5_dr(src8, cv, out_sb, bias_idx):
            for q in range(4):
                ps = ps_dw.tile([128, 8, SW], F32, tag="psdw")
                pp = 0
                for j in range(5):
                    for i0 in (0, 2, 4):
                        w8 = cv * 30 + pp * 2
                        nc.tensor.matmul(
                            ps[:], w_dw8[:, w8:w8 + 2, :],
                            dr_rhs(src8, q, i0, j),
                            start=(pp == 0), stop=(pp == 14), perf_mode=DR)
                        pp += 1
                nc.scalar.activation(out_sb[:, q * 8:(q + 1) * 8, :], ps[:],
                                     AF.Relu, bias=bcol(bias_idx))

        t_kca = work.tile([128, ROWS_T, SW], BF16, tag="t_kca")
        dw5_dr(low8_t, 0, t_kca, 0)

        # ---- KCA chain: 1x1 (32->8) relu, 1x1 (8->32) sigmoid ----
        m1o = work.tile([32, ROWS_T, SW], BF16, tag="m1o")
        for q in range(4):
            ps = ps_pw.tile([32, 8, SW], F32, tag="pspw")
            nc.tensor.matmul(ps[:], w_m1[:], t_kca[:, q * 8:(q + 1) * 8, :],
                             start=True, stop=True)
            nc.scalar.activation(m1o[:, q * 8:(q + 1) * 8, :], ps[:],
                                 AF.Relu, bias=bcol(3, 32))
        kca = work.tile([128, ROWS_T, SW], BF16, tag="kca")
        for q in range(4):
            ps = ps_pw.tile([128, 8, SW], F32, tag="pspw")
            nc.tensor.matmul(ps[:], w_m2[:], m1o[:, q * 8:(q + 1) * 8, :],
                             start=True, stop=True)
            nc.scalar.activation(kca[:, q * 8:(q + 1) * 8, :], ps[:],
                                 AF.Sigmoid, bias=bcol(4))

        # ---- KSA chain (strip pairs: K=64 -> M=100, then K=100 -> M=64) ----
        t_ksa = work.tile([128, ROWS_T, SW], BF16, tag="t_ksa")
        dw5_dr(up8_t, 1, t_ksa, 1)
        k1o = work.tile([100, 2, ROWS_T, SW], BF16, tag="k1o")
        for g in range(2):
            for q in range(4):
                ps = ps_pw.tile([100, 8, SW], F32, tag="pspw")
                nc.tensor.matmul(
                    ps[:], w_k1[g * 64:(g + 1) * 64, :],
                    t_ksa[g * 64:(g + 1) * 64, q * 8:(q + 1) * 8, :],
                    start=True, stop=True)
                nc.scalar.activation(k1o[:, g, q * 8:(q + 1) * 8, :], ps[:],
                                     AF.Relu, bias=bcol(5, 100))
        # ksa laid out [128 = strip*32 + tap, rows, cols] (slots 25-31 pad)
        ksa = work.tile([128, ROWS_T, SW], BF16, tag="ksa")
        for g in range(2):
            for q in range(4):
                ps = ps_pw.tile([64, 8, SW], F32, tag="pspw")
                nc.tensor.matmul(ps[:], w_k2[:],
                                 k1o[:, g, q * 8:(q + 1) * 8, :],
                                 start=True, stop=True)
                nc.scalar.activation(
                    ksa[64 * g:64 * (g + 1), q * 8:(q + 1) * 8, :], ps[:],
                    AF.Sigmoid, bias=bcol(6, 64))

        # ---- dynamic conv + low 1x1, fused: low = sum_t W_low @ m_t ----
        # m_t = (lower_shift * w_dyn[c,t]) * ksa_rep[t];  W_low = kron(I4,
        # (low_pw_w * dyn-scale fold is NOT needed — w_dyn applied in STT)
        low_o = outp.tile([128, ROWS_T, SW], F32, tag="low_o")
        for hf in range(2):
            lps0 = ps_low.tile([128, 8, SW], F32, tag="pslow")
            lps1 = ps_low.tile([128, 8, SW], F32, tag="pslow")
            lps = [lps0, lps1]
            for t, (i, j) in enumerate(TAPS):
                for qq in range(2):
                    q = hf * 2 + qq
                    rep = ps_rep.tile([128, 8, SW], F32, tag="rep")
                    nc.tensor.matmul(rep[:], sel[:, t, :],
                                     ksa[:, q * 8:(q + 1) * 8, :],
                                     start=True, stop=True)
                    mt = work.tile([128, 8, SW], BF16, tag=f"mt{qq}")
                    nc.vector.scalar_tensor_tensor(
                        mt[:], low_t[:, q * 8 + i:q * 8 + i + 8, j:j + SW],
                        wdyn[:, t:t + 1], rep[:], ALU.mult, ALU.mult)
                    nc.tensor.matmul(lps[qq][:], w_low[:], mt[:],
                                     start=(t == 0), stop=(t == 24))
            for qq in range(2):
                q = hf * 2 + qq
                nc.scalar.activation(low_o[:, q * 8:(q + 1) * 8, :],
                                     lps[qq][:], AF.Identity, bias=bcol(7))

        # ---- up branch: dw5 + bias, gate by kca (fused on DVE), 1x1 ----
        gated = work.tile([128, ROWS_T, SW], BF16, tag="gated")
        for q in range(4):
            ps = ps_dw.tile([128, 8, SW], F32, tag="psdw")
            for t, (i, j) in enumerate(TAPS):
                nc.tensor.matmul(
                    ps[:], w_dw[:, 50 + t, :],
                    up_t[:, q * 8 + i:q * 8 + i + 8, j:j + SW],
                    start=(t == 0), stop=(t == 24))
            nc.vector.scalar_tensor_tensor(
                gated[:, q * 8:(q + 1) * 8, :], ps[:], bcol(2),
                kca[:, q * 8:(q + 1) * 8, :], ALU.add, ALU.mult)
        up_o = outp.tile([128, ROWS_T, SW], F32, tag="up_o")
        for q in range(4):
            ps = ps_pw.tile([128, 8, SW], F32, tag="pspw")
            nc.tensor.matmul(ps[:], w_up[:], gated[:, q * 8:(q + 1) * 8, :],
                             start=True, stop=True)
            nc.scalar.activation(up_o[:, q * 8:(q + 1) * 8, :], ps[:],
                                 AF.Identity, bias=bcol(8))

        # ---- store (striped layout; host de-interleaves) ----
        nc.sync.dma_start(low_od[:, r0:r0 + ROWS_T, :], low_o[:])
        nc.sync.dma_start(up_od[:, r0:r0 + ROWS_T, :], up_o[:])


_NC_CACHE = {}


def _build_nc():
    if "nc" in _NC_CACHE:
        return _NC_CACHE["nc"]
    nc = bacc.Bacc("TRN2", target_bir_lowering=False)
    lower_d = nc.dram_tensor("lower_sh", (128, HSH + 4, SW + 4), BF16,
                             kind="ExternalInput")
    upper_d = nc.dram_tensor("upper_sh", (128, HSH + 4, SW + 4), BF16,
                             kind="ExternalInput")
    lower8_d = nc.dram_tensor("lower8_sh", (128, HSH + 5, SW + 4), F8,
                              kind="ExternalInput")
    upper8_d = nc.dram_tensor("upper8_sh", (128, HSH + 5, SW + 4), F8,
                              kind="ExternalInput")
    wdw_d = nc.dram_tensor("w_dw", (128, 75, 128), BF16, kind="ExternalInput")
    wdw8_d = nc.dram_tensor("w_dw8", (128, 60, 128), F8,
                            kind="ExternalInput")
    sel_d = nc.dram_tensor("sel", (128, 25, 128), BF16, kind="ExternalInput")
    wm1_d = nc.dram_tensor("w_m1", (128, 32), BF16, kind="ExternalInput")
    wm2_d = nc.dram_tensor("w_m2", (32, 128), BF16, kind="ExternalInput")
    wk1_d = nc.dram_tensor("w_k1", (128, 100), BF16, kind="ExternalInput")
    wk2_d = nc.dram_tensor("w_k2", (100, 64), BF16, kind="ExternalInput")
    wlow_d = nc.dram_tensor("w_low", (128, 128), BF16, kind="ExternalInput")
    wup_d = nc.dram_tensor("w_up", (128, 128), BF16, kind="ExternalInput")
    wdyn_d = nc.dram_tensor("w_dyn", (128, 25), F32, kind="ExternalInput")
    bias_d = nc.dram_tensor("biases", (128, 9), F32, kind="ExternalInput")
    low_od = nc.dram_tensor("low_out", (128, HSH, SW), F32,
                            kind="ExternalOutput")
    up_od = nc.dram_tensor("up_out", (128, HSH, SW), F32,
                           kind="ExternalOutput")
    io = (lower_d, upper_d, lower8_d, upper8_d, wdw_d, wdw8_d, sel_d, wm1_d,
          wm2_d, wk1_d, wk2_d, wlow_d, wup_d, wdyn_d, bias_d, low_od, up_od)
    with tile.TileContext(nc) as tc:
        with ExitStack() as ctx:
            _emit(ctx, tc, io)
    nc.compile()
    _NC_CACHE["nc"] = nc
    return nc


def _prep_weights(kca_dw_w, kca_dw_b, kca_m1_w, kca_m1_b, kca_m2_w, kca_m2_b,
                  ksa_dw_w, ksa_dw_b, ksa_m1_w, ksa_m1_b, ksa_m2_w, ksa_m2_b,
                  low_dyn_w, low_dyn_b, low_pw_w, low_pw_b,
                  up_dw_w, up_dw_b, up_pw_w, up_pw_b):
    f = np.float32
    import ml_dtypes
    bf = ml_dtypes.bfloat16
    f8 = ml_dtypes.float8_e4m3
    w_dw = np.zeros((128, 75, 128), f)
    ar = np.arange(128)
    for cv, wt in enumerate([kca_dw_w, ksa_dw_w, up_dw_w]):
        w2 = np.asarray(wt, f).reshape(CH, 25)
        for t in range(25):
            w_dw[ar, cv * 25 + t, ar] = np.tile(w2[:, t], 4)
    # DoubleRow fp8 vertical tap-pair diagonals for kca/ksa dw5:
    # per col j, row pairs (0,1), (2,3), (4,zero)
    w_dw8 = np.zeros((128, 60, 128), f)
    for cv, wt in enumerate([kca_dw_w, ksa_dw_w]):
        w3 = np.asarray(wt, f).reshape(CH, 5, 5)  # (c, i, j)
        pp = 0
        for j in range(5):
            for i0 in (0, 2, 4):
                for kt in range(2):
                    if i0 + kt < 5:
                        w_dw8[ar, cv * 30 + pp * 2 + kt, ar] = \
                            np.tile(w3[:, i0 + kt, j], 4)
                pp += 1
    sel = np.zeros((128, 25, 128), f)
    for s in range(4):
        for t in range(25):
            sel[s * 32 + t, t, s * 32:(s + 1) * 32] = 1.0
    i4, i2 = np.eye(4, dtype=f), np.eye(2, dtype=f)
    w_m1 = np.kron(i4, np.asarray(kca_m1_w, f).T)        # (128, 32)
    w_m2 = np.kron(i4, np.asarray(kca_m2_w, f).T)        # (32, 128)
    w_k1 = np.kron(i2, np.asarray(ksa_m1_w, f).T)        # (64, 100)
    w_k1 = np.vstack([w_k1, w_k1])                       # (128, 100) dup
    w_k2 = np.zeros((100, 64), f)                        # padded to 32-slots
    w2t = np.asarray(ksa_m2_w, f).T                      # (50, 25)
    for sl in range(2):
        w_k2[sl * 50:(sl + 1) * 50, sl * 32:sl * 32 + 25] = w2t
    w_low = np.kron(i4, np.asarray(low_pw_w, f).T)       # (128, 128)
    w_up = np.kron(i4, np.asarray(up_pw_w, f).T)         # (128, 128)
    w_dyn = np.tile(np.asarray(low_dyn_w, f).reshape(CH, 25), (4, 1))
    bias = np.zeros((128, 9), f)
    bias[:, 0] = np.tile(np.asarray(kca_dw_b, f), 4)
    bias[:, 1] = np.tile(np.asarray(ksa_dw_b, f), 4)
    bias[:, 2] = np.tile(np.asarray(up_dw_b, f), 4)
    bias[:32, 3] = np.tile(np.asarray(kca_m1_b, f), 4)
    bias[:, 4] = np.tile(np.asarray(kca_m2_b, f), 4)
    bias[:100, 5] = np.tile(np.asarray(ksa_m1_b, f), 2)
    for sl in range(2):
        bias[sl * 32:sl * 32 + 25, 6] = np.asarray(ksa_m2_b, f)
    b_low = np.asarray(low_pw_w, f) @ np.asarray(low_dyn_b, f).reshape(CH) \
        + np.asarray(low_pw_b, f)
    bias[:, 7] = np.tile(b_low, 4)
    bias[:, 8] = np.tile(np.asarray(up_pw_b, f), 4)
    return dict(w_dw=w_dw.astype(bf), w_dw8=w_dw8.astype(f8),
                sel=sel.astype(bf),
                w_m1=w_m1.astype(bf), w_m2=w_m2.astype(bf),
                w_k1=w_k1.astype(bf), w_k2=w_k2.astype(bf),
                w_low=w_low.astype(bf), w_up=w_up.astype(bf),
                w_dyn=w_dyn, biases=bias)


def kernel(lower, upper, **wts):
    global LAST_EXEC_NS
    import ml_dtypes
    bf = ml_dtypes.bfloat16
    nc = _build_nc()
    wmap = _prep_weights(**wts)
    lp = np.pad(np.ascontiguousarray(np.asarray(lower, np.float32)),
                ((0, 0), (0, 0), (2, 2), (2, 2))).astype(bf)
    up = np.pad(np.ascontiguousarray(np.asarray(upper, np.float32)),
                ((0, 0), (0, 0), (2, 2), (2, 2))).astype(bf)

    def stripe(x, dt):
        # (32, 132, 260) -> (128 = strip*32+c, 132, 68), strips overlap by 4
        out = np.empty((128, HSH + 4, SW + 4), dt)
        for s in range(4):
            out[s * 32:(s + 1) * 32] = x[:, :, s * SW:s * SW + SW + 4]
        return out

    f8 = ml_dtypes.float8_e4m3
    in_maps = []
    for k in range(N_CORES):
        n, half = k // 2, k % 2
        m = dict(wmap)
        ls = lp[n, :, half * HSH:half * HSH + HSH + 4, :]
        us = up[n, :, half * HSH:half * HSH + HSH + 4, :]
        m["lower_sh"] = stripe(ls, bf)
        m["upper_sh"] = stripe(us, bf)

        def pad8(a):
            # one extra zero row for the (tap-row-4, zero) DR pseudo-pairs
            out = np.zeros((128, HSH + 5, SW + 4), f8)
            out[:, :HSH + 4] = a.astype(f8)
            return out

        m["lower8_sh"] = pad8(m["lower_sh"])
        m["upper8_sh"] = pad8(m["upper_sh"])
        in_maps.append(m)
    trace = os.environ.get("BASS_KERNEL_TRACE", "0") == "1"
    res = run_bass_kernel_spmd(nc, in_maps, core_ids=list(range(N_CORES)),
                               trace=trace)
    LAST_EXEC_NS = res.exec_time_ns
    low = np.empty((NB, CH, H, W), np.float32)
    upo = np.empty((NB, CH, H, W), np.float32)
    for k in range(N_CORES):
        n, half = k // 2, k % 2
        for s in range(4):
            low[n, :, half * HSH:(half + 1) * HSH, s * SW:(s + 1) * SW] = \
                res.results[k]["low_out"][s * 32:(s + 1) * 32]
            upo[n, :, half * HSH:(half + 1) * HSH, s * SW:(s + 1) * SW] = \
                res.results[k]["up_out"][s * 32:(s + 1) * 32]
    return low, upo

